# revision 5
# baseline (speedup 1.0000x reference)
"""Multi-head causal attention (B=4, S=2048, D=1024, H=16) for 8 Trainium2 cores.

Sharding: core c = (batch b = c//2, head-group g = c%2). Each core computes,
for its batch and its 8 heads: QKV projections, causal softmax attention, and
a partial output projection (its heads' rows of Wo). Host sums the two
head-group partials per batch and adds the output bias.

v2 layout (all-bf16 pipeline, transposed PV):
 - All inputs arrive bf16; every matmul runs bf16 (1 cycle/row at any N).
 - Scores computed transposed, ST[k, q] = K Q^T, exp'd on ACT into bf16
   pt tiles; causal mask applied post-exp as a DVE multiply with a 0/1
   triangle on the single 128-col diagonal subchunk (exact 128-granular
   trims elsewhere, no mask matmuls on PE).
 - PV computed TRANSPOSED: ctx^T[q, d] = pt^T V per 128-q subchunk with
   M=128 (q), K=128 (keys), N=65 (V plus a ones column) -> ~full PE
   utilization and the softmax denominator lands per-q-partition, so
   normalization is a per-partition tensor_scalar multiply (no partition
   broadcast, no intra-SBUF DMAs).
 - Normalized ctx^T is transposed back per 128x128 tile on the PE (identity
   rhs) for the output projection, whose PSUM result DMAs straight to DRAM.
"""

import sys

if "/opt/trn_rl_repo" not in sys.path:
    sys.path.insert(0, "/opt/trn_rl_repo")

import numpy as np
import ml_dtypes

B, S, D = 4, 2048, 1024
H, DH = 16, 64
NCORES = 8
GH = H // 2            # heads per core
GW = GH * DH           # head-group width (512)
NP = GW // 128         # head pairs per core (4)
SM_SCALE = float(1.0 / np.sqrt(np.float32(D)))

# build-time tuning knobs (swept via TimelineSim)
CFG = {
    "schr_cycle": ["a"],   # exp engine per late non-diag chunk (ACT only)
    "pt_bufs": 6,
    "late_qt2": True,
    "late_qt1": False,
    "ys_act": True,
    "filler_pace": 1,
    "tp_tag": "sm",
    "mask_pool": False,
    "norm_act": False,
    "copy_q": "a",
    "copy_k": "a",
    "copy_v": "a",
    "copy_by_st": ["a", "d", "d", "a"],
    "split_i": False,
    "ys_by_qt": ["a", "a", "a", "a"],
    "safe_barriers": True,
    "order4": ["2g0", "a1", "2g1", "3g1", "3g2", "2g2", "3g3"],
}


def build_mha_kernel(S_, D_, debug=False, debug_taps=False):
    import concourse.bass as bass  # noqa: F401
    import concourse.mybir as mybir
    import concourse.tile as tile
    from concourse import bacc

    f32 = mybir.dt.float32
    bf16 = mybir.dt.bfloat16

    KT = D_ // 128          # input-dim tiles
    NQT = S_ // 512         # q tiles
    NST = S_ // 512         # s tiles for streaming XT in phase 1
    NKC = S_ // 128         # key chunks

    nc = bacc.Bacc("TRN2", target_bir_lowering=False, debug=debug)

    XT_d = nc.dram_tensor("XT", [D_, S_], bf16, kind="ExternalInput")
    WQ_d = nc.dram_tensor("WQ", [D_, GW], bf16, kind="ExternalInput")
    WK_d = nc.dram_tensor("WK", [D_, GW], bf16, kind="ExternalInput")
    WV_d = nc.dram_tensor("WV", [D_, GW], bf16, kind="ExternalInput")
    WO_d = nc.dram_tensor("WO", [GW, D_], bf16, kind="ExternalInput")
    M1_d = nc.dram_tensor("M1", [128, 128], bf16, kind="ExternalInput")
    ID_d = nc.dram_tensor("ID", [128, 128], bf16, kind="ExternalInput")
    ON_d = nc.dram_tensor("ON", [128, NKC * GH], bf16, kind="ExternalInput")
    Y_d = nc.dram_tensor("Y", [S_, D_], bf16, kind="ExternalOutput")
    if debug_taps:
        QTD = nc.dram_tensor("QTD", [128, NP, S_], bf16, kind="ExternalOutput")
        KTD = nc.dram_tensor("KTD", [128, NP, S_], bf16, kind="ExternalOutput")
        VD = nc.dram_tensor("VD", [128, NKC, GH * 65], bf16,
                            kind="ExternalOutput")
        PTD = nc.dram_tensor("PTD", [128, 2, 512], bf16, kind="ExternalOutput")
        CXD = nc.dram_tensor("CXD", [128, 2, 512], f32, kind="ExternalOutput")
        RECD = nc.dram_tensor("RECD", [128, 2, 4], f32, kind="ExternalOutput")
        CTD = nc.dram_tensor("CTD", [128, 4, 128], bf16, kind="ExternalOutput")
        CND = nc.dram_tensor("CND", [128, 512], bf16, kind="ExternalOutput")

    Exp = mybir.ActivationFunctionType.Exp

    with tile.TileContext(nc) as tc:
        with tc.tile_pool(name="const", bufs=1) as const_pool, \
             tc.tile_pool(name="big", bufs=1) as big_pool, \
             tc.tile_pool(name="xw", bufs=1) as xw_pool, \
             tc.tile_pool(name="att", bufs=CFG["pt_bufs"]) as att_pool, \
             tc.tile_pool(name="nrm", bufs=2) as nrm_pool, \
             tc.tile_pool(name="ps", bufs=1, space="PSUM") as ps_pool:

            # ---- persistent activations ----
            QT_t = big_pool.tile([128, NP, S_], bf16)     # Q^T  [dout, s]
            KT_t = big_pool.tile([128, NP, S_], bf16)     # K^T  [dout, s]
            V_t = big_pool.tile([128, NKC, GH * 65], bf16)  # V + ones col/head

            WQ_t = xw_pool.tile([128, KT, GW], bf16, tag="wq")
            WK_t = xw_pool.tile([128, KT, GW], bf16, tag="wk")
            WV_t = xw_pool.tile([128, KT, GW], bf16, tag="wv")
            XT_r = XT_d.rearrange("(kt p) s -> p kt s", p=128)
            WQ_r = WQ_d.rearrange("(kt p) n -> p kt n", p=128)
            # first s-tile + per-chunk WQ DMAs issued first so the first
            # matmuls can start as soon as possible
            xt0 = xw_pool.tile([128, KT, 512], bf16, tag="xt", bufs=2)
            WK_r = WK_d.rearrange("(kt p) n -> p kt n", p=128)
            WV_r = WV_d.rearrange("(kt p) n -> p kt n", p=128)
            # split the first loads so the first psqk matmuls (which need
            # xt0 + WQ) can start after half the serialized DMA transfers
            nc.sync.dma_start(xt0[:, 0:KT // 2], XT_r[:, 0:KT // 2, 0:512])
            nc.sync.dma_start(WQ_t[:, 0:KT // 2], WQ_r[:, 0:KT // 2])
            nc.sync.dma_start(xt0[:, KT // 2:], XT_r[:, KT // 2:, 0:512])
            nc.sync.dma_start(WQ_t[:, KT // 2:], WQ_r[:, KT // 2:])
            nc.sync.dma_start(WK_t, WK_r)
            nc.sync.dma_start(WV_t, WV_r)
            # consts needed from attention onward
            m1t = const_pool.tile([128, 128], bf16)  # M1[r,c] = 1 iff r <= c
            nc.sync.dma_start(m1t, M1_d[:])
            ident = const_pool.tile([128, 128], bf16)
            nc.sync.dma_start(ident, ID_d[:])
            WO_t = const_pool.tile([128, NP, D_], bf16)
            nc.sync.dma_start(WO_t, WO_d.rearrange("(c p) n -> p c n", p=128))
            nc.sync.dma_start(
                V_t.rearrange("p kc (h e) -> p (kc h) e", e=65)[:, :, 64:65],
                ON_d[:, :, None])

            NOUT = max(1, D_ // 512)
            OW = min(512, D_)

            def emit_qkv_block(st, xt, bi):
                """One of 12 QKV sub-blocks for s-tile st (8 psqk + 4 psv)."""
                if bi < 8:
                    c, which = bi // 2, ("q", "k")[bi % 2]
                    wt, outt = ((WQ_t, QT_t), (WK_t, KT_t))[bi % 2]
                    psqk = ps_pool.tile([128, 512], f32, tag="sm", bufs=2,
                                        name=f"psqk_{st}_{c}_{which}")
                    for kt in range(KT):
                        nc.tensor.matmul(
                            psqk[:, :],
                            lhsT=wt[:, kt, c * 128:(c + 1) * 128],
                            rhs=xt[:, kt, :],
                            start=(kt == 0), stop=(kt == KT - 1))
                    ceng = CFG["copy_by_st"][min(st, 3)]
                    if ceng == "d":
                        nc.vector.tensor_copy(
                            out=outt[:, c, st * 512:(st + 1) * 512],
                            in_=psqk[:, :])
                    else:
                        nc.scalar.copy(
                            out=outt[:, c, st * 512:(st + 1) * 512],
                            in_=psqk[:, :])
                else:
                    sc = bi - 8
                    scc = st * 4 + sc
                    psv = ps_pool.tile([128, 512], f32, tag="sm", bufs=2,
                                       name=f"psv_{st}_{sc}")
                    for kt in range(KT):
                        nc.tensor.matmul(
                            psv[:, :],
                            lhsT=xt[:, kt, sc * 128:(sc + 1) * 128],
                            rhs=WV_t[:, kt, :],
                            start=(kt == 0), stop=(kt == KT - 1))
                    if CFG["copy_by_st"][min(st, 3)] == "d":
                        nc.vector.tensor_copy(
                            out=V_t[:, scc].rearrange(
                                "p (h e) -> p h e", e=65)[:, :, 0:64],
                            in_=psv[:, :].rearrange("p (h d) -> p h d", d=64))
                    else:
                        nc.scalar.copy(
                            out=V_t[:, scc].rearrange(
                                "p (h e) -> p h e", e=65)[:, :, 0:64],
                            in_=psv[:, :].rearrange("p (h d) -> p h d", d=64))

            def emit_oproj_block(qt, ctxn, m, last=False):
                """Output projection for q-subchunk m of q-tile qt. For the
                final block of the program, stage and DMA in two half-row
                pieces so the copy and DMA overlap in the drain."""
                qs = qt * 512
                yp = ps_pool.tile([128, 2, 512], f32, tag="stp", bufs=2,
                                  name=f"yp_{qt}_{m}")
                for n in range(NOUT):
                    for c in range(NP):
                        nc.tensor.matmul(
                            yp[:, n, 0:OW],
                            lhsT=ctxn[c][:, m * 128:(m + 1) * 128],
                            rhs=WO_t[:, c, n * OW:(n + 1) * OW],
                            start=(c == 0), stop=(c == NP - 1),
                            skip_group_check=True)
                ys = nrm_pool.tile([128, 2, 512], bf16, tag="ys",
                                   bufs=3, name=f"ys_{qt}_{m}")
                ys_eng = CFG["ys_by_qt"][min(qt, 3)]
                if last and NOUT == 2:
                    nc.scalar.copy(out=ys[:, 0, 0:OW], in_=yp[:, 0, 0:OW])
                    nc.sync.dma_start(
                        Y_d[qs + m * 128:qs + (m + 1) * 128, 0:OW],
                        ys[:, 0, 0:OW])
                    nc.vector.tensor_copy(out=ys[:, 1, 0:OW],
                                          in_=yp[:, 1, 0:OW])
                    nc.sync.dma_start(
                        Y_d[qs + m * 128:qs + (m + 1) * 128, OW:2 * OW],
                        ys[:, 1, 0:OW])
                    return
                if ys_eng == "a":
                    nc.scalar.copy(out=ys[:, 0:NOUT, 0:OW],
                                   in_=yp[:, 0:NOUT, 0:OW])
                else:
                    nc.vector.tensor_copy(out=ys[:, 0:NOUT, 0:OW],
                                          in_=yp[:, 0:NOUT, 0:OW])
                nc.sync.dma_start(
                    Y_d[qs + m * 128:qs + (m + 1) * 128, :],
                    ys[:, 0:NOUT, 0:OW])

            # ---- PE filler FIFO: QKV sub-blocks and output-projection
            # blocks are queued here and drained one per attention chunk,
            # between a chunk's exp dispatch and its PV matmuls, so the PE
            # always has independent work while ACT crunches the exp. ----
            fifo = []          # entries: (key or None, emit_fn)
            queued = set()     # (st, bi) keys of pending QKV blocks
            fill_ctr = {"n": 0}

            def _pop_one():
                key, fn = fifo.pop(0)
                if key is not None:
                    queued.discard(key)
                fn()

            def filler(k=1):
                if k == 1:
                    fill_ctr["n"] += 1
                    if fill_ctr["n"] % CFG["filler_pace"] != 0:
                        return
                for _ in range(min(k, len(fifo))):
                    _pop_one()

            def require_qkv(st, q_only=False):
                """Drain the filler FIFO (in order) until every QKV block of
                s-tile st (or just its Q-projection blocks) has been emitted;
                called at attention-block entry so reads never precede their
                writers in the stream."""
                want = {(st, bi) for bi in (range(0, 8, 2) if q_only
                                            else range(12))}
                while want & queued:
                    _pop_one()

            def require_blocks(st, bis):
                """In-order drain until specific QKV blocks of s-tile st have
                been emitted (exact per-chunk dependencies)."""
                want = {(st, bi) for bi in bis}
                while want & queued:
                    _pop_one()

            def queue_qkv(st):
                if st == 0:
                    xt = xt0
                else:
                    xt = xw_pool.tile([128, KT, 512], bf16, tag="xt", bufs=2,
                                      name=f"xt_{st}")
                    nc.sync.dma_start(xt, XT_r[:, :, st * 512:(st + 1) * 512])
                for bi in range(12):
                    fifo.append(((st, bi),
                                 lambda st=st, xt=xt, bi=bi:
                                 emit_qkv_block(st, xt, bi)))
                    queued.add((st, bi))

            def queue_oproj(qt, ctxn):
                for m in range(4):
                    fifo.append((None,
                                 lambda qt=qt, ctxn=ctxn, m=m:
                                 emit_oproj_block(qt, ctxn, m)))

            # bf16 Schraudolph exp: bitcast(int16(x*A + B)) ~= e^x with a
            # ~1.7% rms sawtooth error. The softmax bias cancels and the
            # error is only used on a minority of key chunks, keeping the
            # end-to-end error well under the tolerance while letting DVE
            # and Pool share the exp stream with ACT.
            SCHR_A = float(128.0 / np.log(2.0)) * SM_SCALE
            # truncate-toward-zero convert: center the sawtooth with C=5.5
            SCHR_B = 16256.0 - 5.5
            i16 = mybir.dt.int16
            schr_cycle = list(CFG["schr_cycle"])
            schr_state = {"n": 0}

            def emit_scores_exp(qt, c, kc, name, late=False):
                """Transposed scores + exp for one 128-key chunk; returns the
                bf16 probability tile pt (masked on the diagonal chunk)."""
                jp = kc - 4 * qt
                trim = 128 * jp if jp >= 0 else 0
                qs = qt * 512
                if CFG["safe_barriers"]:
                    require_blocks(qt, (2 * c,))             # Q(qt, c)
                    require_blocks(kc // 4, (2 * c + 1,))    # K(st, c)
                    require_blocks(kc // 4, (8 + kc % 4,))   # V chunk
                stp = ps_pool.tile([128, 2, 512], f32, tag="stp",
                                   bufs=2, name=f"stp_{name}")
                for i in (0, 1):
                    nc.tensor.matmul(
                        stp[:, i, trim:512],
                        lhsT=KT_t[64 * i:64 * i + 64, c,
                                  kc * 128:(kc + 1) * 128],
                        rhs=QT_t[64 * i:64 * i + 64, c, qs + trim:qs + 512],
                        start=True, stop=True, skip_group_check=True)
                pt = att_pool.tile([128, 2, 512], bf16, tag="pt",
                                   name=f"pt_{name}")
                eng = "a"
                if late and jp < 0:
                    eng = schr_cycle[schr_state["n"] % len(schr_cycle)]
                    schr_state["n"] += 1
                if late and jp < 0 and CFG["split_i"]:
                    # split the two head-halves across ACT (exact exp) and
                    # DVE (Schraudolph): both run concurrently, halving the
                    # chunk's exp latency and ACT's per-chunk load
                    nc.scalar.activation(
                        pt[:, 0, trim:512], stp[:, 0, trim:512],
                        Exp, scale=SM_SCALE)
                    nc.vector.tensor_scalar(
                        pt.bitcast(i16)[:, 1, trim:512],
                        stp[:, 1, trim:512],
                        SCHR_A, SCHR_B,
                        mybir.AluOpType.mult, mybir.AluOpType.add)
                elif eng == "a":
                    nc.scalar.activation(
                        pt[:, :, trim:512], stp[:, :, trim:512],
                        Exp, scale=SM_SCALE)
                else:
                    engine = nc.vector
                    engine.tensor_scalar(
                        pt.bitcast(i16)[:, :, trim:512],
                        stp[:, :, trim:512],
                        SCHR_A, SCHR_B,
                        mybir.AluOpType.mult, mybir.AluOpType.add)
                if jp >= 0:
                    # zero the masked triangle of the diagonal 128-col
                    # subchunk (cols beyond it are fully unmasked; cols
                    # before it were trimmed)
                    meng = nc.gpsimd if CFG["mask_pool"] else nc.vector
                    for i in (0, 1):
                        meng.tensor_mul(
                            pt[:, i, trim:trim + 128],
                            pt[:, i, trim:trim + 128], m1t)
                return pt

            def emit_pv(ctxPS, pt, qt, c, kc, kc_first, kc_last):
                """Accumulate transposed PV for one key chunk into ctxPS."""
                jp = kc - 4 * qt
                for m in range(max(jp, 0), 4):
                    for i in (0, 1):
                        h = 2 * c + i
                        # start only on the first matmul touching each
                        # i-bank: its start poisons the whole 2KB zero
                        # region, so the sibling m-groups' first writes
                        # zero-initialize themselves
                        nc.tensor.matmul(
                            ctxPS[:, i, m * 65:m * 65 + 65],
                            lhsT=pt[:, i, m * 128:(m + 1) * 128],
                            rhs=V_t[:, kc, h * 65:(h + 1) * 65],
                            start=(kc == kc_first and m == max(jp, 0)),
                            stop=(jp == m or kc == kc_last),
                            skip_group_check=True)

            def emit_norm_tp(src, rec, ctxnT, ctxn_c, m, name, sbuf=False):
                """Normalize ctx^T subchunk m from src (PSUM or SBUF view
                [128, 2, >=260] in (i, m*65+e) layout) and transpose it into
                ctxn_c. Pool handles the multiplies when src is SBUF (it
                cannot touch PSUM on hardware)."""
                nc.vector.reciprocal(rec[:, :, m], src[:, :, m * 65 + 64])
                for i in (0, 1):
                    if CFG["norm_act"]:
                        nc.scalar.mul(
                            ctxnT[:, m, i * 64:(i + 1) * 64],
                            src[:, i, m * 65:m * 65 + 64],
                            rec[:, i, m:m + 1])
                    else:
                        nc.vector.tensor_scalar_mul(
                            ctxnT[:, m, i * 64:(i + 1) * 64],
                            src[:, i, m * 65:m * 65 + 64],
                            rec[:, i, m:m + 1])
                tp = ps_pool.tile([128, 128], bf16, tag=CFG["tp_tag"],
                                  bufs=2, name=f"tp_{name}")
                nc.tensor.transpose(tp, ctxnT[:, m, :], ident)
                nc.vector.tensor_copy(
                    out=ctxn_c[:, m * 128:(m + 1) * 128], in_=tp)

            def emit_att(qt, late=False):
                """Whole attention for q-tile qt (PSUM-resident ctx
                accumulation), returning ctxn tiles for the oproj.

                Software-pipelined one chunk ahead across the flattened
                (c, kc) sequence: scores(n+1) is emitted before PV(n), so
                PV's exp dependency has a full chunk of slack and the
                c-boundary ctxPS handoff is absorbed by the lag."""
                nkc = 4 * qt + 4
                ctxn = [nrm_pool.tile([128, 512], bf16, tag=f"ctxn{c}",
                                      bufs=3, name=f"ctxn{c}_{qt}")
                        for c in range(NP)]
                states = {}

                def cstate(c):
                    if c not in states:
                        # per-(i) halves: slice [:, i, m*65:(m+1)*65] holds
                        # the m-th q-subchunk of ctx^T (64 d + denominator)
                        states[c] = (
                            ps_pool.tile([128, 2, 512], f32, tag="ctx",
                                         bufs=1, name=f"ctxPS_{qt}_{c}"),
                            nrm_pool.tile([128, 2, 4], f32, tag="rec",
                                          name=f"rec_{qt}_{c}"),
                            nrm_pool.tile([128, 4, 128], bf16, tag="ctxnT",
                                          name=f"ctxnT_{qt}_{c}"))
                    return states[c]

                def chunk_tail(c, kc, pt):
                    ctxPS, rec, ctxnT = cstate(c)
                    emit_pv(ctxPS, pt, qt, c, kc, 0, nkc - 1)
                    filler()
                    if kc - 4 * qt >= 0:
                        emit_norm_tp(ctxPS, rec, ctxnT, ctxn[c],
                                     kc - 4 * qt, f"{qt}_{c}_{kc - 4 * qt}")

                prev = None
                for c in range(NP):
                    for kc in range(nkc):
                        pt = emit_scores_exp(qt, c, kc, f"{qt}_{c}_{kc}",
                                             late=late)
                        if prev is not None:
                            chunk_tail(*prev)
                        prev = (c, kc, pt)
                chunk_tail(*prev)
                return ctxn

            def emit_attL_group(qt, g, ctxACC, state):
                """Attention for q-tile qt restricted to key chunks
                [4g, 4g+4), accumulated into SBUF ctxACC between groups so
                the group can be scheduled early (as soon as QKV for s-tiles
                qt and g are done). Software-pipelined across the flattened
                (c, kc) sequence like emit_att."""
                ctxPSs = {}

                def chunk_tail(c, kc, pt):
                    if c not in ctxPSs:
                        ctxPSs[c] = ps_pool.tile(
                            [128, 2, 512], f32, tag="ctx", bufs=1,
                            name=f"ctxPS_{qt}_{c}_g{g}")
                    ctxPS = ctxPSs[c]
                    emit_pv(ctxPS, pt, qt, c, kc, 4 * g, 4 * g + 3)
                    filler()
                    if kc == 4 * g + 3:
                        if g == 0:
                            nc.vector.tensor_copy(
                                out=ctxACC[c][:, :, 0:260],
                                in_=ctxPS[:, :, 0:260])
                        else:
                            nc.vector.tensor_add(ctxACC[c][:, :, 0:260],
                                                 ctxACC[c][:, :, 0:260],
                                                 ctxPS[:, :, 0:260])

                prev = None
                for c in range(NP):
                    for kc in range(4 * g, 4 * g + 4):
                        pt = emit_scores_exp(qt, c, kc, f"{qt}_{c}_{kc}",
                                             late=(g >= 1))
                        if prev is not None:
                            chunk_tail(*prev)
                        prev = (c, kc, pt)
                chunk_tail(*prev)
                if g == qt:
                    # final (diagonal) group: normalize + transpose with the
                    # last q-tile's output projection interleaved per
                    # m-subchunk so the tail drains early
                    recs = [nrm_pool.tile([128, 2, 4], f32, tag=f"recL{c}",
                                          bufs=1, name=f"rec_{qt}_{c}")
                            for c in range(NP)]
                    ctxnTs = [nrm_pool.tile([128, 4, 128], bf16,
                                            tag=f"ctxnTL{c}", bufs=1,
                                            name=f"ctxnT_{qt}_{c}")
                              for c in range(NP)]
                    for m in range(4):
                        for c in range(NP):
                            filler()
                            emit_norm_tp(ctxACC[c], recs[c], ctxnTs[c],
                                         state["ctxn"][c], m, f"{qt}_{c}_{m}",
                                         sbuf=True)
                        emit_oproj_block(qt, state["ctxn"], m,
                                         last=(m == 3
                                               and state.get("last_prog")))

            # ---- schedule ----
            # The last q-tile L has (L+1)x the exp work of q-tile 0 and its
            # exp stream gates the activation engine; give it its queries
            # early (QKV(L) right after QKV(0)) and spread its key-chunk
            # groups across the timeline so ACT never runs dry or backs up.
            L = NQT - 1
            if L == 0:
                queue_qkv(0)
                filler(99)
                ctxn0 = emit_att(0)
                for m in range(4):
                    emit_oproj_block(0, ctxn0, m)
            else:
                ctxACC = [big_pool.tile([128, 2, 512], f32,
                                        name=f"ctxACC{c}")
                          for c in range(NP)]
                ctxnL = [nrm_pool.tile([128, 512], bf16, tag=f"ctxn{c}",
                                       bufs=3, name=f"ctxn{c}_L")
                         for c in range(NP)]
                stateL = {"ctxn": ctxnL, "last_prog": True}
                if False:
                    # split BOTH qt=2 and qt=3 into 4-kc groups and spread
                    # them so no contiguous ACT-bound exp blob remains
                    ctxACC2 = [big_pool.tile([128, 2, 512], f32,
                                             name=f"ctxACC2_{c}")
                               for c in range(NP)]
                    ctxn2 = [nrm_pool.tile([128, 512], bf16, tag=f"ctxn{c}",
                                           bufs=3, name=f"ctxn{c}_q2")
                             for c in range(NP)]
                    state2 = {"ctxn": ctxn2, "last_prog": False}
                    queue_qkv(0)   # drained just-in-time by att(0)'s chunks
                    queue_qkv(3)   # Q of the last q-tile, ready early
                    ctxn0 = emit_att(0)
                    queue_oproj(0, ctxn0)
                    queue_qkv(2)
                    emit_attL_group(3, 0, ctxACC, stateL)
                    queue_qkv(1)

                    def A2(g):
                        emit_attL_group(2, g, ctxACC2, state2)

                    def A3(g):
                        emit_attL_group(3, g, ctxACC, stateL)

                    def A1():
                        ctxn1 = emit_att(1, late=CFG["late_qt1"])
                        queue_oproj(1, ctxn1)

                    for tok in CFG["order4"]:
                        {"a1": A1, "2g0": lambda: A2(0), "2g1": lambda: A2(1),
                         "2g2": lambda: A2(2), "3g1": lambda: A3(1),
                         "3g2": lambda: A3(2), "3g3": lambda: A3(3)}[tok]()
                    filler(99)
                else:
                    queue_qkv(0)   # drained just-in-time by att(0)'s chunks
                    queue_qkv(L)   # Q of the last q-tile, ready early
                    ctxn_prev = emit_att(0)
                    queue_oproj(0, ctxn_prev)
                    for st in range(1, L):
                        queue_qkv(st)
                        emit_attL_group(L, st - 1, ctxACC, stateL)
                        ctxn_prev = emit_att(
                            st, late=((st >= 2 and CFG["late_qt2"])
                                      or (st == 1 and CFG["late_qt1"])))
                        queue_oproj(st, ctxn_prev)
                    emit_attL_group(L, L - 1, ctxACC, stateL)
                    emit_attL_group(L, L, ctxACC, stateL)
                    filler(99)

    nc.compile()
    return nc


_NC_CACHE = {}


def _get_nc():
    key = (S, D)
    if key not in _NC_CACHE:
        _NC_CACHE[key] = build_mha_kernel(S, D)
    return _NC_CACHE[key]


def make_consts(S_):
    bf = ml_dtypes.bfloat16
    r = np.arange(128)
    m1 = (r[:, None] <= r[None, :]).astype(bf)       # M1[r,c] = r <= c
    ident = np.eye(128, dtype=bf)
    on = np.ones((128, S_ // 128 * GH), dtype=bf)
    return m1, ident, on


def shard_inputs(X, Wq, Wk, Wv, Wo):
    """Build the 8 per-core input maps from full inputs."""
    bf = ml_dtypes.bfloat16
    X = np.asarray(X, dtype=np.float32)
    Wq = np.asarray(Wq, dtype=np.float32)
    Wk = np.asarray(Wk, dtype=np.float32)
    Wv = np.asarray(Wv, dtype=np.float32)
    Wo = np.asarray(Wo, dtype=np.float32)
    m1, ident, on = make_consts(S)
    in_maps = []
    for c in range(NCORES):
        b, g = c // 2, c % 2
        in_maps.append({
            "XT": np.ascontiguousarray(X[b].T).astype(bf),
            "WQ": np.ascontiguousarray(Wq[:, g * GW:(g + 1) * GW]).astype(bf),
            "WK": np.ascontiguousarray(Wk[:, g * GW:(g + 1) * GW]).astype(bf),
            "WV": np.ascontiguousarray(Wv[:, g * GW:(g + 1) * GW]).astype(bf),
            "WO": np.ascontiguousarray(Wo[g * GW:(g + 1) * GW, :]).astype(bf),
            "M1": m1, "ID": ident, "ON": on,
        })
    return in_maps


def kernel(X, Wq, Wk, Wv, Wo, bo):
    from concourse.bass_utils import run_bass_kernel_spmd

    nc = _get_nc()
    in_maps = shard_inputs(X, Wq, Wk, Wv, Wo)
    res = run_bass_kernel_spmd(nc, in_maps, core_ids=list(range(NCORES)))
    bo = np.asarray(bo, dtype=np.float32)
    Y = np.empty((B, S, D), dtype=np.float32)
    for b in range(B):
        Y[b] = (res.results[2 * b]["Y"].astype(np.float32)
                + res.results[2 * b + 1]["Y"].astype(np.float32) + bo)
    return Y


# revision 6
# speedup vs baseline: 1.0000x; 1.0000x over previous
"""Multi-head causal attention (B=4, S=2048, D=1024, H=16) for 8 Trainium2 cores.

Sharding: core c = (batch b = c//2, head-group g = c%2). Each core computes,
for its batch and its 8 heads: QKV projections, causal softmax attention, and
a partial output projection (its heads' rows of Wo). Host sums the two
head-group partials per batch and adds the output bias.

v2 layout (all-bf16 pipeline, transposed PV):
 - All inputs arrive bf16; every matmul runs bf16 (1 cycle/row at any N).
 - Scores computed transposed, ST[k, q] = K Q^T, exp'd on ACT into bf16
   pt tiles; causal mask applied post-exp as a DVE multiply with a 0/1
   triangle on the single 128-col diagonal subchunk (exact 128-granular
   trims elsewhere, no mask matmuls on PE).
 - PV computed TRANSPOSED: ctx^T[q, d] = pt^T V per 128-q subchunk with
   M=128 (q), K=128 (keys), N=65 (V plus a ones column) -> ~full PE
   utilization and the softmax denominator lands per-q-partition, so
   normalization is a per-partition tensor_scalar multiply (no partition
   broadcast, no intra-SBUF DMAs).
 - Normalized ctx^T is transposed back per 128x128 tile on the PE (identity
   rhs) for the output projection, whose PSUM result DMAs straight to DRAM.
"""

import sys

if "/opt/trn_rl_repo" not in sys.path:
    sys.path.insert(0, "/opt/trn_rl_repo")

import numpy as np
import ml_dtypes

B, S, D = 4, 2048, 1024
H, DH = 16, 64
NCORES = 8
GH = H // 2            # heads per core
GW = GH * DH           # head-group width (512)
NP = GW // 128         # head pairs per core (4)
SM_SCALE = float(1.0 / np.sqrt(np.float32(D)))

# build-time tuning knobs (swept via TimelineSim)
CFG = {
    "schr_cycle": ["a"],   # exp engine per late non-diag chunk (ACT only)
    "pt_bufs": 6,
    "late_qt2": True,
    "late_qt1": False,
    "ys_act": True,
    "filler_pace": 1,
    "tp_tag": "sm",
    "mask_pool": False,
    "norm_act": False,
    "copy_q": "a",
    "copy_k": "a",
    "copy_v": "a",
    "copy_by_st": ["a", "d", "d", "a"],
    "split_i": False,
    "ys_by_qt": ["a", "a", "a", "a"],
    "safe_barriers": True,
    "order4": ["2g0", "a1", "2g1", "3g1", "3g2", "2g2", "3g3"],
}


def build_mha_kernel(S_, D_, debug=False, debug_taps=False):
    import concourse.bass as bass  # noqa: F401
    import concourse.mybir as mybir
    import concourse.tile as tile
    from concourse import bacc

    f32 = mybir.dt.float32
    bf16 = mybir.dt.bfloat16

    KT = D_ // 128          # input-dim tiles
    NQT = S_ // 512         # q tiles
    NST = S_ // 512         # s tiles for streaming XT in phase 1
    NKC = S_ // 128         # key chunks

    nc = bacc.Bacc("TRN2", target_bir_lowering=False, debug=debug)

    XT_d = nc.dram_tensor("XT", [D_, S_], bf16, kind="ExternalInput")
    WQ_d = nc.dram_tensor("WQ", [D_, GW], bf16, kind="ExternalInput")
    WK_d = nc.dram_tensor("WK", [D_, GW], bf16, kind="ExternalInput")
    WV_d = nc.dram_tensor("WV", [D_, GW], bf16, kind="ExternalInput")
    WO_d = nc.dram_tensor("WO", [GW, D_], bf16, kind="ExternalInput")
    M1_d = nc.dram_tensor("M1", [128, 128], bf16, kind="ExternalInput")
    ID_d = nc.dram_tensor("ID", [128, 128], bf16, kind="ExternalInput")
    ON_d = nc.dram_tensor("ON", [128, NKC * GH], bf16, kind="ExternalInput")
    Y_d = nc.dram_tensor("Y", [S_, D_], bf16, kind="ExternalOutput")
    if debug_taps:
        QTD = nc.dram_tensor("QTD", [128, NP, S_], bf16, kind="ExternalOutput")
        KTD = nc.dram_tensor("KTD", [128, NP, S_], bf16, kind="ExternalOutput")
        VD = nc.dram_tensor("VD", [128, NKC, GH * 65], bf16,
                            kind="ExternalOutput")
        PTD = nc.dram_tensor("PTD", [128, 2, 512], bf16, kind="ExternalOutput")
        CXD = nc.dram_tensor("CXD", [128, 2, 512], f32, kind="ExternalOutput")
        RECD = nc.dram_tensor("RECD", [128, 2, 4], f32, kind="ExternalOutput")
        CTD = nc.dram_tensor("CTD", [128, 4, 128], bf16, kind="ExternalOutput")
        CND = nc.dram_tensor("CND", [128, 512], bf16, kind="ExternalOutput")

    Exp = mybir.ActivationFunctionType.Exp

    with tile.TileContext(nc) as tc:
        with tc.tile_pool(name="const", bufs=1) as const_pool, \
             tc.tile_pool(name="big", bufs=1) as big_pool, \
             tc.tile_pool(name="xw", bufs=1) as xw_pool, \
             tc.tile_pool(name="att", bufs=CFG["pt_bufs"]) as att_pool, \
             tc.tile_pool(name="nrm", bufs=2) as nrm_pool, \
             tc.tile_pool(name="ps", bufs=1, space="PSUM") as ps_pool:

            # ---- persistent activations ----
            QT_t = big_pool.tile([128, NP, S_], bf16)     # Q^T  [dout, s]
            KT_t = big_pool.tile([128, NP, S_], bf16)     # K^T  [dout, s]
            V_t = big_pool.tile([128, NKC, GH * 65], bf16)  # V + ones col/head

            WQ_t = xw_pool.tile([128, KT, GW], bf16, tag="wq")
            WK_t = xw_pool.tile([128, KT, GW], bf16, tag="wk")
            WV_t = xw_pool.tile([128, KT, GW], bf16, tag="wv")
            XT_r = XT_d.rearrange("(kt p) s -> p kt s", p=128)
            WQ_r = WQ_d.rearrange("(kt p) n -> p kt n", p=128)
            # first s-tile + per-chunk WQ DMAs issued first so the first
            # matmuls can start as soon as possible
            xt0 = xw_pool.tile([128, KT, 512], bf16, tag="xt", bufs=2)
            WK_r = WK_d.rearrange("(kt p) n -> p kt n", p=128)
            WV_r = WV_d.rearrange("(kt p) n -> p kt n", p=128)
            # split the first loads so the first psqk matmuls (which need
            # xt0 + WQ) can start after half the serialized DMA transfers
            nc.sync.dma_start(xt0[:, 0:KT // 2], XT_r[:, 0:KT // 2, 0:512])
            nc.sync.dma_start(WQ_t[:, 0:KT // 2], WQ_r[:, 0:KT // 2])
            nc.sync.dma_start(xt0[:, KT // 2:], XT_r[:, KT // 2:, 0:512])
            nc.sync.dma_start(WQ_t[:, KT // 2:], WQ_r[:, KT // 2:])
            nc.sync.dma_start(WK_t, WK_r)
            nc.sync.dma_start(WV_t, WV_r)
            # consts needed from attention onward
            m1t = const_pool.tile([128, 128], bf16)  # M1[r,c] = 1 iff r <= c
            nc.sync.dma_start(m1t, M1_d[:])
            ident = const_pool.tile([128, 128], bf16)
            nc.sync.dma_start(ident, ID_d[:])
            WO_t = const_pool.tile([128, NP, D_], bf16)
            nc.sync.dma_start(WO_t, WO_d.rearrange("(c p) n -> p c n", p=128))
            nc.sync.dma_start(
                V_t.rearrange("p kc (h e) -> p (kc h) e", e=65)[:, :, 64:65],
                ON_d[:, :, None])

            NOUT = max(1, D_ // 512)
            OW = min(512, D_)

            def emit_qkv_block(st, xt, bi):
                """One of 12 QKV sub-blocks for s-tile st (8 psqk + 4 psv)."""
                if bi < 8:
                    c, which = bi // 2, ("q", "k")[bi % 2]
                    wt, outt = ((WQ_t, QT_t), (WK_t, KT_t))[bi % 2]
                    psqk = ps_pool.tile([128, 512], f32, tag="sm", bufs=2,
                                        name=f"psqk_{st}_{c}_{which}")
                    for kt in range(KT):
                        nc.tensor.matmul(
                            psqk[:, :],
                            lhsT=wt[:, kt, c * 128:(c + 1) * 128],
                            rhs=xt[:, kt, :],
                            start=(kt == 0), stop=(kt == KT - 1))
                    ceng = CFG["copy_by_st"][min(st, 3)]
                    if ceng == "d":
                        nc.vector.tensor_copy(
                            out=outt[:, c, st * 512:(st + 1) * 512],
                            in_=psqk[:, :])
                    else:
                        nc.scalar.copy(
                            out=outt[:, c, st * 512:(st + 1) * 512],
                            in_=psqk[:, :])
                else:
                    sc = bi - 8
                    scc = st * 4 + sc
                    psv = ps_pool.tile([128, 512], f32, tag="sm", bufs=2,
                                       name=f"psv_{st}_{sc}")
                    for kt in range(KT):
                        nc.tensor.matmul(
                            psv[:, :],
                            lhsT=xt[:, kt, sc * 128:(sc + 1) * 128],
                            rhs=WV_t[:, kt, :],
                            start=(kt == 0), stop=(kt == KT - 1))
                    if CFG["copy_by_st"][min(st, 3)] == "d":
                        nc.vector.tensor_copy(
                            out=V_t[:, scc].rearrange(
                                "p (h e) -> p h e", e=65)[:, :, 0:64],
                            in_=psv[:, :].rearrange("p (h d) -> p h d", d=64))
                    else:
                        nc.scalar.copy(
                            out=V_t[:, scc].rearrange(
                                "p (h e) -> p h e", e=65)[:, :, 0:64],
                            in_=psv[:, :].rearrange("p (h d) -> p h d", d=64))

            def emit_oproj_block(qt, ctxn, m, last=False):
                """Output projection for q-subchunk m of q-tile qt. For the
                final block of the program, stage and DMA in two half-row
                pieces so the copy and DMA overlap in the drain."""
                qs = qt * 512
                yp = ps_pool.tile([128, 2, 512], f32, tag="stp", bufs=2,
                                  name=f"yp_{qt}_{m}")
                for n in range(NOUT):
                    for c in range(NP):
                        nc.tensor.matmul(
                            yp[:, n, 0:OW],
                            lhsT=ctxn[c][:, m * 128:(m + 1) * 128],
                            rhs=WO_t[:, c, n * OW:(n + 1) * OW],
                            start=(c == 0), stop=(c == NP - 1),
                            skip_group_check=True)
                ys = nrm_pool.tile([128, 2, 512], bf16, tag="ys",
                                   bufs=3, name=f"ys_{qt}_{m}")
                ys_eng = CFG["ys_by_qt"][min(qt, 3)]
                if last and NOUT == 2:
                    nc.scalar.copy(out=ys[:, 0, 0:OW], in_=yp[:, 0, 0:OW])
                    nc.sync.dma_start(
                        Y_d[qs + m * 128:qs + (m + 1) * 128, 0:OW],
                        ys[:, 0, 0:OW])
                    nc.vector.tensor_copy(out=ys[:, 1, 0:OW],
                                          in_=yp[:, 1, 0:OW])
                    nc.sync.dma_start(
                        Y_d[qs + m * 128:qs + (m + 1) * 128, OW:2 * OW],
                        ys[:, 1, 0:OW])
                    return
                if ys_eng == "a":
                    nc.scalar.copy(out=ys[:, 0:NOUT, 0:OW],
                                   in_=yp[:, 0:NOUT, 0:OW])
                else:
                    nc.vector.tensor_copy(out=ys[:, 0:NOUT, 0:OW],
                                          in_=yp[:, 0:NOUT, 0:OW])
                nc.sync.dma_start(
                    Y_d[qs + m * 128:qs + (m + 1) * 128, :],
                    ys[:, 0:NOUT, 0:OW])

            # ---- PE filler FIFO: QKV sub-blocks and output-projection
            # blocks are queued here and drained one per attention chunk,
            # between a chunk's exp dispatch and its PV matmuls, so the PE
            # always has independent work while ACT crunches the exp. ----
            fifo = []          # entries: (key or None, emit_fn)
            queued = set()     # (st, bi) keys of pending QKV blocks
            fill_ctr = {"n": 0}

            def _pop_one():
                key, fn = fifo.pop(0)
                if key is not None:
                    queued.discard(key)
                fn()

            def filler(k=1):
                if k == 1:
                    fill_ctr["n"] += 1
                    if fill_ctr["n"] % CFG["filler_pace"] != 0:
                        return
                for _ in range(min(k, len(fifo))):
                    _pop_one()

            def require_qkv(st, q_only=False):
                """Drain the filler FIFO (in order) until every QKV block of
                s-tile st (or just its Q-projection blocks) has been emitted;
                called at attention-block entry so reads never precede their
                writers in the stream."""
                want = {(st, bi) for bi in (range(0, 8, 2) if q_only
                                            else range(12))}
                while want & queued:
                    _pop_one()

            def require_blocks(st, bis):
                """In-order drain until specific QKV blocks of s-tile st have
                been emitted (exact per-chunk dependencies)."""
                want = {(st, bi) for bi in bis}
                while want & queued:
                    _pop_one()

            def queue_qkv(st):
                if st == 0:
                    xt = xt0
                else:
                    xt = xw_pool.tile([128, KT, 512], bf16, tag="xt", bufs=2,
                                      name=f"xt_{st}")
                    nc.sync.dma_start(xt, XT_r[:, :, st * 512:(st + 1) * 512])
                for bi in range(12):
                    fifo.append(((st, bi),
                                 lambda st=st, xt=xt, bi=bi:
                                 emit_qkv_block(st, xt, bi)))
                    queued.add((st, bi))

            def queue_oproj(qt, ctxn):
                for m in range(4):
                    fifo.append((None,
                                 lambda qt=qt, ctxn=ctxn, m=m:
                                 emit_oproj_block(qt, ctxn, m)))

            # bf16 Schraudolph exp: bitcast(int16(x*A + B)) ~= e^x with a
            # ~1.7% rms sawtooth error. The softmax bias cancels and the
            # error is only used on a minority of key chunks, keeping the
            # end-to-end error well under the tolerance while letting DVE
            # and Pool share the exp stream with ACT.
            SCHR_A = float(128.0 / np.log(2.0)) * SM_SCALE
            # truncate-toward-zero convert: center the sawtooth with C=5.5
            SCHR_B = 16256.0 - 5.5
            i16 = mybir.dt.int16
            schr_cycle = list(CFG["schr_cycle"])
            schr_state = {"n": 0}

            def emit_scores_exp(qt, c, kc, name, late=False):
                """Transposed scores + exp for one 128-key chunk; returns the
                bf16 probability tile pt (masked on the diagonal chunk)."""
                jp = kc - 4 * qt
                trim = 128 * jp if jp >= 0 else 0
                qs = qt * 512
                if CFG["safe_barriers"]:
                    require_blocks(qt, (2 * c,))             # Q(qt, c)
                    require_blocks(kc // 4, (2 * c + 1,))    # K(st, c)
                    require_blocks(kc // 4, (8 + kc % 4,))   # V chunk
                stp = ps_pool.tile([128, 2, 512], f32, tag="stp",
                                   bufs=2, name=f"stp_{name}")
                for i in (0, 1):
                    nc.tensor.matmul(
                        stp[:, i, trim:512],
                        lhsT=KT_t[64 * i:64 * i + 64, c,
                                  kc * 128:(kc + 1) * 128],
                        rhs=QT_t[64 * i:64 * i + 64, c, qs + trim:qs + 512],
                        start=True, stop=True, skip_group_check=True)
                pt = att_pool.tile([128, 2, 512], bf16, tag="pt",
                                   name=f"pt_{name}")
                eng = "a"
                if late and jp < 0:
                    eng = schr_cycle[schr_state["n"] % len(schr_cycle)]
                    schr_state["n"] += 1
                if late and jp < 0 and CFG["split_i"]:
                    # split the two head-halves across ACT (exact exp) and
                    # DVE (Schraudolph): both run concurrently, halving the
                    # chunk's exp latency and ACT's per-chunk load
                    nc.scalar.activation(
                        pt[:, 0, trim:512], stp[:, 0, trim:512],
                        Exp, scale=SM_SCALE)
                    nc.vector.tensor_scalar(
                        pt.bitcast(i16)[:, 1, trim:512],
                        stp[:, 1, trim:512],
                        SCHR_A, SCHR_B,
                        mybir.AluOpType.mult, mybir.AluOpType.add)
                elif eng == "a":
                    nc.scalar.activation(
                        pt[:, :, trim:512], stp[:, :, trim:512],
                        Exp, scale=SM_SCALE)
                else:
                    engine = nc.vector
                    engine.tensor_scalar(
                        pt.bitcast(i16)[:, :, trim:512],
                        stp[:, :, trim:512],
                        SCHR_A, SCHR_B,
                        mybir.AluOpType.mult, mybir.AluOpType.add)
                if jp >= 0:
                    # zero the masked triangle of the diagonal 128-col
                    # subchunk (cols beyond it are fully unmasked; cols
                    # before it were trimmed)
                    meng = nc.gpsimd if CFG["mask_pool"] else nc.vector
                    for i in (0, 1):
                        meng.tensor_mul(
                            pt[:, i, trim:trim + 128],
                            pt[:, i, trim:trim + 128], m1t)
                return pt

            def emit_pv(ctxPS, pt, qt, c, kc, kc_first, kc_last):
                """Accumulate transposed PV for one key chunk into ctxPS."""
                jp = kc - 4 * qt
                for m in range(max(jp, 0), 4):
                    for i in (0, 1):
                        h = 2 * c + i
                        # start only on the first matmul touching each
                        # i-bank: its start poisons the whole 2KB zero
                        # region, so the sibling m-groups' first writes
                        # zero-initialize themselves
                        nc.tensor.matmul(
                            ctxPS[:, i, m * 65:m * 65 + 65],
                            lhsT=pt[:, i, m * 128:(m + 1) * 128],
                            rhs=V_t[:, kc, h * 65:(h + 1) * 65],
                            start=(kc == kc_first and m == max(jp, 0)),
                            stop=(jp == m or kc == kc_last),
                            skip_group_check=True)

            def emit_norm_tp(src, rec, ctxnT, ctxn_c, m, name, sbuf=False):
                """Normalize ctx^T subchunk m from src (PSUM or SBUF view
                [128, 2, >=260] in (i, m*65+e) layout) and transpose it into
                ctxn_c. Pool handles the multiplies when src is SBUF (it
                cannot touch PSUM on hardware)."""
                nc.vector.reciprocal(rec[:, :, m], src[:, :, m * 65 + 64])
                for i in (0, 1):
                    if CFG["norm_act"]:
                        nc.scalar.mul(
                            ctxnT[:, m, i * 64:(i + 1) * 64],
                            src[:, i, m * 65:m * 65 + 64],
                            rec[:, i, m:m + 1])
                    else:
                        nc.vector.tensor_scalar_mul(
                            ctxnT[:, m, i * 64:(i + 1) * 64],
                            src[:, i, m * 65:m * 65 + 64],
                            rec[:, i, m:m + 1])
                tp = ps_pool.tile([128, 128], bf16, tag=CFG["tp_tag"],
                                  bufs=2, name=f"tp_{name}")
                nc.tensor.transpose(tp, ctxnT[:, m, :], ident)
                nc.vector.tensor_copy(
                    out=ctxn_c[:, m * 128:(m + 1) * 128], in_=tp)

            def emit_att(qt, late=False):
                """Whole attention for q-tile qt (PSUM-resident ctx
                accumulation), returning ctxn tiles for the oproj.

                Software-pipelined one chunk ahead across the flattened
                (c, kc) sequence: scores(n+1) is emitted before PV(n), so
                PV's exp dependency has a full chunk of slack and the
                c-boundary ctxPS handoff is absorbed by the lag."""
                nkc = 4 * qt + 4
                ctxn = [nrm_pool.tile([128, 512], bf16, tag=f"ctxn{c}",
                                      bufs=3, name=f"ctxn{c}_{qt}")
                        for c in range(NP)]
                states = {}

                def cstate(c):
                    if c not in states:
                        # per-(i) halves: slice [:, i, m*65:(m+1)*65] holds
                        # the m-th q-subchunk of ctx^T (64 d + denominator)
                        states[c] = (
                            ps_pool.tile([128, 2, 512], f32, tag="ctx",
                                         bufs=1, name=f"ctxPS_{qt}_{c}"),
                            nrm_pool.tile([128, 2, 4], f32, tag="rec",
                                          name=f"rec_{qt}_{c}"),
                            nrm_pool.tile([128, 4, 128], bf16, tag="ctxnT",
                                          name=f"ctxnT_{qt}_{c}"))
                    return states[c]

                def chunk_tail(c, kc, pt):
                    ctxPS, rec, ctxnT = cstate(c)
                    emit_pv(ctxPS, pt, qt, c, kc, 0, nkc - 1)
                    filler()
                    if kc - 4 * qt >= 0:
                        emit_norm_tp(ctxPS, rec, ctxnT, ctxn[c],
                                     kc - 4 * qt, f"{qt}_{c}_{kc - 4 * qt}")

                prev = None
                for c in range(NP):
                    for kc in range(nkc):
                        pt = emit_scores_exp(qt, c, kc, f"{qt}_{c}_{kc}",
                                             late=late)
                        if prev is not None:
                            chunk_tail(*prev)
                        prev = (c, kc, pt)
                chunk_tail(*prev)
                return ctxn

            def emit_attL_group(qt, g, ctxACC, state):
                """Attention for q-tile qt restricted to key chunks
                [4g, 4g+4), accumulated into SBUF ctxACC between groups so
                the group can be scheduled early (as soon as QKV for s-tiles
                qt and g are done). Software-pipelined across the flattened
                (c, kc) sequence like emit_att."""
                ctxPSs = {}

                def chunk_tail(c, kc, pt):
                    if c not in ctxPSs:
                        ctxPSs[c] = ps_pool.tile(
                            [128, 2, 512], f32, tag="ctx", bufs=1,
                            name=f"ctxPS_{qt}_{c}_g{g}")
                    ctxPS = ctxPSs[c]
                    emit_pv(ctxPS, pt, qt, c, kc, 4 * g, 4 * g + 3)
                    filler()
                    if kc == 4 * g + 3:
                        if g == 0:
                            nc.vector.tensor_copy(
                                out=ctxACC[c][:, :, 0:260],
                                in_=ctxPS[:, :, 0:260])
                        else:
                            nc.vector.tensor_add(ctxACC[c][:, :, 0:260],
                                                 ctxACC[c][:, :, 0:260],
                                                 ctxPS[:, :, 0:260])

                prev = None
                for c in range(NP):
                    for kc in range(4 * g, 4 * g + 4):
                        pt = emit_scores_exp(qt, c, kc, f"{qt}_{c}_{kc}",
                                             late=(g >= 1))
                        if prev is not None:
                            chunk_tail(*prev)
                        prev = (c, kc, pt)
                chunk_tail(*prev)
                if g == qt:
                    # final (diagonal) group: batch the reciprocals up-front,
                    # then per m-subchunk emit all DVE multiplies before the
                    # PE transposes so the transpose never waits on the
                    # per-(c,m) normalize chain; the last q-tile's output
                    # projection stays interleaved per m so the tail drains
                    recs = [nrm_pool.tile([128, 2, 4], f32, tag=f"recL{c}",
                                          bufs=1, name=f"rec_{qt}_{c}")
                            for c in range(NP)]
                    ctxnTs = [nrm_pool.tile([128, 4, 128], bf16,
                                            tag=f"ctxnTL{c}", bufs=1,
                                            name=f"ctxnT_{qt}_{c}")
                              for c in range(NP)]
                    for c in range(NP):
                        nc.vector.reciprocal(
                            recs[c][:, :, :],
                            ctxACC[c][:, :, 64::65][:, :, 0:4])
                    for m in range(4):
                        for c in range(NP):
                            for i in (0, 1):
                                nc.vector.tensor_scalar_mul(
                                    ctxnTs[c][:, m, i * 64:(i + 1) * 64],
                                    ctxACC[c][:, i, m * 65:m * 65 + 64],
                                    recs[c][:, i, m:m + 1])
                        for c in range(NP):
                            filler()
                            tp = ps_pool.tile([128, 128], bf16,
                                              tag=CFG["tp_tag"], bufs=2,
                                              name=f"tpf_{qt}_{c}_{m}")
                            nc.tensor.transpose(tp, ctxnTs[c][:, m, :], ident)
                            nc.vector.tensor_copy(
                                out=state["ctxn"][c][:, m * 128:(m + 1) * 128],
                                in_=tp)
                        emit_oproj_block(qt, state["ctxn"], m,
                                         last=(m == 3
                                               and state.get("last_prog")))

            # ---- schedule ----
            # The last q-tile L has (L+1)x the exp work of q-tile 0 and its
            # exp stream gates the activation engine; give it its queries
            # early (QKV(L) right after QKV(0)) and spread its key-chunk
            # groups across the timeline so ACT never runs dry or backs up.
            L = NQT - 1
            if L == 0:
                queue_qkv(0)
                filler(99)
                ctxn0 = emit_att(0)
                for m in range(4):
                    emit_oproj_block(0, ctxn0, m)
            else:
                ctxACC = [big_pool.tile([128, 2, 512], f32,
                                        name=f"ctxACC{c}")
                          for c in range(NP)]
                ctxnL = [nrm_pool.tile([128, 512], bf16, tag=f"ctxn{c}",
                                       bufs=3, name=f"ctxn{c}_L")
                         for c in range(NP)]
                stateL = {"ctxn": ctxnL, "last_prog": True}
                if False:
                    # split BOTH qt=2 and qt=3 into 4-kc groups and spread
                    # them so no contiguous ACT-bound exp blob remains
                    ctxACC2 = [big_pool.tile([128, 2, 512], f32,
                                             name=f"ctxACC2_{c}")
                               for c in range(NP)]
                    ctxn2 = [nrm_pool.tile([128, 512], bf16, tag=f"ctxn{c}",
                                           bufs=3, name=f"ctxn{c}_q2")
                             for c in range(NP)]
                    state2 = {"ctxn": ctxn2, "last_prog": False}
                    queue_qkv(0)   # drained just-in-time by att(0)'s chunks
                    queue_qkv(3)   # Q of the last q-tile, ready early
                    ctxn0 = emit_att(0)
                    queue_oproj(0, ctxn0)
                    queue_qkv(2)
                    emit_attL_group(3, 0, ctxACC, stateL)
                    queue_qkv(1)

                    def A2(g):
                        emit_attL_group(2, g, ctxACC2, state2)

                    def A3(g):
                        emit_attL_group(3, g, ctxACC, stateL)

                    def A1():
                        ctxn1 = emit_att(1, late=CFG["late_qt1"])
                        queue_oproj(1, ctxn1)

                    for tok in CFG["order4"]:
                        {"a1": A1, "2g0": lambda: A2(0), "2g1": lambda: A2(1),
                         "2g2": lambda: A2(2), "3g1": lambda: A3(1),
                         "3g2": lambda: A3(2), "3g3": lambda: A3(3)}[tok]()
                    filler(99)
                else:
                    queue_qkv(0)   # drained just-in-time by att(0)'s chunks
                    queue_qkv(L)   # Q of the last q-tile, ready early
                    ctxn_prev = emit_att(0)
                    queue_oproj(0, ctxn_prev)
                    for st in range(1, L):
                        queue_qkv(st)
                        emit_attL_group(L, st - 1, ctxACC, stateL)
                        ctxn_prev = emit_att(
                            st, late=((st >= 2 and CFG["late_qt2"])
                                      or (st == 1 and CFG["late_qt1"])))
                        queue_oproj(st, ctxn_prev)
                    emit_attL_group(L, L - 1, ctxACC, stateL)
                    emit_attL_group(L, L, ctxACC, stateL)
                    filler(99)

    nc.compile()
    return nc


_NC_CACHE = {}


def _get_nc():
    key = (S, D)
    if key not in _NC_CACHE:
        _NC_CACHE[key] = build_mha_kernel(S, D)
    return _NC_CACHE[key]


def make_consts(S_):
    bf = ml_dtypes.bfloat16
    r = np.arange(128)
    m1 = (r[:, None] <= r[None, :]).astype(bf)       # M1[r,c] = r <= c
    ident = np.eye(128, dtype=bf)
    on = np.ones((128, S_ // 128 * GH), dtype=bf)
    return m1, ident, on


def shard_inputs(X, Wq, Wk, Wv, Wo):
    """Build the 8 per-core input maps from full inputs."""
    bf = ml_dtypes.bfloat16
    X = np.asarray(X, dtype=np.float32)
    Wq = np.asarray(Wq, dtype=np.float32)
    Wk = np.asarray(Wk, dtype=np.float32)
    Wv = np.asarray(Wv, dtype=np.float32)
    Wo = np.asarray(Wo, dtype=np.float32)
    m1, ident, on = make_consts(S)
    in_maps = []
    for c in range(NCORES):
        b, g = c // 2, c % 2
        in_maps.append({
            "XT": np.ascontiguousarray(X[b].T).astype(bf),
            "WQ": np.ascontiguousarray(Wq[:, g * GW:(g + 1) * GW]).astype(bf),
            "WK": np.ascontiguousarray(Wk[:, g * GW:(g + 1) * GW]).astype(bf),
            "WV": np.ascontiguousarray(Wv[:, g * GW:(g + 1) * GW]).astype(bf),
            "WO": np.ascontiguousarray(Wo[g * GW:(g + 1) * GW, :]).astype(bf),
            "M1": m1, "ID": ident, "ON": on,
        })
    return in_maps


def kernel(X, Wq, Wk, Wv, Wo, bo):
    from concourse.bass_utils import run_bass_kernel_spmd

    nc = _get_nc()
    in_maps = shard_inputs(X, Wq, Wk, Wv, Wo)
    res = run_bass_kernel_spmd(nc, in_maps, core_ids=list(range(NCORES)))
    bo = np.asarray(bo, dtype=np.float32)
    Y = np.empty((B, S, D), dtype=np.float32)
    for b in range(B):
        Y[b] = (res.results[2 * b]["Y"].astype(np.float32)
                + res.results[2 * b + 1]["Y"].astype(np.float32) + bo)
    return Y
